# revision 35
# baseline (speedup 1.0000x reference)
"""Trainium2 Bass kernel for nn_PairwisePredictionHead.

Math (reference):
  xd = x @ W_down.T + b_down             # [L, 128]
  q, k = xd[:, :64], xd[:, 64:]
  h[i,j,:] = W1p @ (q_j*k_i) + W1d @ (q_j - k_i) + b1    # [L, L, 128]
  g = gelu_exact(h)
  out = W2 @ LN(g) + b2                   # [L, L, 64]

Sharding: row-shard i across 8 cores (96 rows each). Each core gets the full
q-side (all 768 j) plus its own 96 k-rows; cores are independent (no
collectives), outputs concatenated on host.

Host precomputes the (tiny) downprojection q/k and per-row gelu bias
b1c = b1 - W1d@k_i; the device runs only the pairwise part.

Device (per core, per i; all matmuls bf16, 512/256-col splits for PSUM banks;
1-deep software pipeline so mm1(i+1) fills the PE while ACT/DVE finish i):
  - lhsT_i = [[W1pT * k_i[:,None]] ; W1dT]  (top half rebuilt per i on DVE)
  - p1[h, j]   = lhsT_i.T @ [q.T; q.T]            (PE, N=768)
  - g  = Gelu(p1 + b1c_i)                         (ACT, bf16 out)
  - g2 = g*g                                      (DVE, bf16)
  - pA[0:66, j] = [W2z.T*ln_g | ones | 0].T @ g   (PE; row 64 = sum_h g)
  - pA[64:66,j] += [0 | ones].T @ g2              (PE; row 65 = sum_h g^2)
  - copy pA[0:66] -> o_sb bf16 (DVE cols 0:CSPL, ACT the rest), batched DMA

Host tail (vectorized numpy): mu = Sg/128, var = Sg2/128 - mu^2,
  r = rsqrt(var+eps), out[i,j,:] = dev_out[:,i,j]*r + (W2@ln_b + b2).
W2z rows are zero-meaned so the matmul absorbs LN's mean subtraction
(w.(g-mu) == (w-mean(w)).g).
"""

import os
from contextlib import ExitStack

import numpy as np
import ml_dtypes

import concourse.bass as bass
import concourse.mybir as mybir
import concourse.tile as tile
from concourse import bacc
from concourse.bass_utils import run_bass_kernel_spmd

F32 = mybir.dt.float32
BF16 = mybir.dt.bfloat16
ALU = mybir.AluOpType
AF = mybir.ActivationFunctionType

B, L, D = 1, 768, 1024
DP, H, NB = 128, 128, 64
NCORES = 8
ROWS = L // NCORES  # 96 pair-grid rows per core
P = 128
EPS = 1e-5


def _build(nc):
    q65in = nc.dram_tensor("q65in", [65, L], BF16, kind="ExternalInput")
    kTin = nc.dram_tensor("kTin", [64, ROWS], F32, kind="ExternalInput")
    b1cTin = nc.dram_tensor("b1cTin", [1, ROWS * P], BF16,
                            kind="ExternalInput")
    W1pT = nc.dram_tensor("W1pT", [64, P], BF16, kind="ExternalInput")
    W1dT = nc.dram_tensor("W1dT", [64, P], BF16, kind="ExternalInput")
    W2A = nc.dram_tensor("W2A", [P, 66], BF16, kind="ExternalInput")
    onesc = nc.dram_tensor("onesc", [P, 2], BF16, kind="ExternalInput")
    # partition-major: rows 0:64 = W2z@g, row 64 = sum(g), row 65 = sum(g^2)
    dev_out = nc.dram_tensor("dev_out", [66, ROWS, L], BF16,
                             kind="ExternalOutput")

    with tile.TileContext(nc) as tc, ExitStack() as ctx:
        const = ctx.enter_context(tc.tile_pool(name="const", bufs=1))
        work = ctx.enter_context(tc.tile_pool(name="work", bufs=6))
        pp1 = ctx.enter_context(tc.tile_pool(name="pp1", bufs=2, space="PSUM"))
        ppA = ctx.enter_context(tc.tile_pool(name="ppA", bufs=2, space="PSUM"))

        # ---- constants into SBUF ----
        q65 = const.tile([65, L], BF16)
        nc.sync.dma_start(out=q65, in_=q65in[:])
        kT_sb = const.tile([64, ROWS], F32)
        nc.sync.dma_start(out=kT_sb, in_=kTin[:])
        # gelu bias rows, staged on partition 64 so they can be copied into
        # the stationary tiles' bias row (row 64) by same-partition DVE ops
        b1cT_sb = const.tile([65, ROWS, P], BF16)
        nc.sync.dma_start(
            out=b1cT_sb[64:65, :, :],
            in_=b1cTin[:].rearrange("o (r p) -> o r p", p=P))
        W1pT_sb = const.tile([64, P], BF16)
        nc.sync.dma_start(out=W1pT_sb, in_=W1pT[:])
        W1dT_sb = const.tile([64, P], BF16)
        nc.sync.dma_start(out=W1dT_sb, in_=W1dT[:])
        W2A_sb = const.tile([P, 66], BF16)
        nc.sync.dma_start(out=W2A_sb, in_=W2A[:])
        ones_sb = const.tile([P, 2], BF16)
        nc.sync.dma_start(out=ones_sb, in_=onesc[:])

        # K=65 stationary tiles: rows 0:64 = W1pT*diag(k_i) + W1dT,
        # row 64 = gelu bias b1c_i (contracted against the ones row of q65)
        lhsT_t = [const.tile([65, P], BF16, tag=f"lhsT{t}", name=f"lhsT{t}")
                  for t in range(2)]

        # ---- main loop (1-deep software pipeline) ----
        DB = 8      # i's per output-DMA batch
        CSPL = 464  # psum->SBUF copy column split (DVE | ACT)
        o_sb = None

        def build_lt(i):
            lt = lhsT_t[i % 2]
            nc.vector.scalar_tensor_tensor(
                lt[0:64, :], W1pT_sb, kT_sb[:, i:i + 1], W1dT_sb,
                ALU.mult, ALU.add)
            nc.vector.tensor_copy(lt[64:65, :], b1cT_sb[64:65, i, :])
            return lt

        def mm1(i, lt):
            p1 = pp1.tile([P, L], F32, tag="p1", name="p1")
            nc.tensor.matmul(p1[:, 0:512], lt, q65[:, 0:512],
                             start=True, stop=True)
            nc.tensor.matmul(p1[:, 512:768], lt, q65[:, 512:768],
                             start=True, stop=True)
            return p1

        def copy_out(pc, ci):
            # lagged psum->SBUF copy of iteration ci (runs during ci+1 so
            # ACT/DVE never stall waiting on that iteration's mm2)
            nonlocal o_sb
            bi = ci % DB
            if bi == 0:
                o_sb = work.tile([66, DB, L], BF16, tag="osb", name="osb")
            nc.vector.tensor_copy(o_sb[:, bi, 0:CSPL], pc[0:66, 0:CSPL])
            nc.scalar.copy(o_sb[:, bi, CSPL:L], pc[0:66, CSPL:L])
            if bi == DB - 1:
                i0 = ci - (DB - 1)
                nc.sync.dma_start(out=dev_out[:, i0:i0 + DB, :], in_=o_sb)

        lt = build_lt(0)
        p1 = mm1(0, lt)
        prev = None
        for ii in range(ROWS):
            g = work.tile([P, L], BF16, tag="g", name="g")
            nc.scalar.activation(g, p1, AF.Gelu)

            if ii + 1 < ROWS:
                lt = build_lt(ii + 1)
                p1 = mm1(ii + 1, lt)

            g2 = work.tile([P, L], BF16, tag="g2", name="g2")
            nc.vector.tensor_mul(g2, g, g)

            if prev is not None:
                copy_out(*prev)

            pA = ppA.tile([P, L], F32, tag="pA", name="pA")
            # MM-A first (needs only g): W2A = [W2z*ln_g | ones | zeros]
            # fills rows 0:66, with sum(g) on row 64 and row 65 zeroed.
            # MM-B then accumulates sum(g^2) into row 65 (start=False onto
            # MM-A's closed accumulation group; row 64 += 0).
            nc.tensor.matmul(pA[0:66, 0:512], W2A_sb, g[:, 0:512],
                             start=True, stop=True)
            nc.tensor.matmul(pA[0:66, 512:768], W2A_sb, g[:, 512:768],
                             start=True, stop=True)
            nc.tensor.matmul(pA[64:66, 0:512], ones_sb, g2[:, 0:512],
                             start=False, stop=True, skip_group_check=True)
            nc.tensor.matmul(pA[64:66, 512:768], ones_sb, g2[:, 512:768],
                             start=False, stop=True, skip_group_check=True)

            prev = (pA, ii)
        copy_out(*prev)


def host_prep(x, W_down, b_down, W1, b1, ln_g, ln_b, W2, b2):
    f32 = np.float32
    bf16 = ml_dtypes.bfloat16
    # downprojection on host (tiny): xd = x @ W_down.T + b_down
    xd = x[0].astype(f32) @ W_down.astype(f32).T + b_down.astype(f32)
    qv = xd[:, :64]        # [L, 64]
    kv = xd[:, 64:]        # [L, 64]
    q65f = np.concatenate([qv.T, np.ones((1, L))], axis=0)  # [65, L]
    W1d = W1[:, 64:].astype(f32)
    # b1c[:, i] = b1 - W1d @ k_i
    b1c = b1.astype(f32)[:, None] - W1d @ kv.T.astype(f32)  # [128, L]

    W2g = W2.astype(np.float64) * ln_g.astype(np.float64)[None, :]
    W2z = W2g - W2g.mean(axis=1, keepdims=True)  # zero-mean rows absorb LN mu
    W2A = np.concatenate([W2z.T, np.ones((P, 1)), np.zeros((P, 1))],
                         axis=1)  # [128, 66]
    common = {
        "q65in": np.ascontiguousarray(q65f.astype(bf16)),
        "W1pT": np.ascontiguousarray(W1[:, :64].T.astype(bf16)),
        "W1dT": np.ascontiguousarray(W1[:, 64:].T.astype(bf16)),
        "W2A": np.ascontiguousarray(W2A.astype(bf16)),
        "onesc": np.ascontiguousarray(
            np.concatenate([np.zeros((P, 1)), np.ones((P, 1))],
                           axis=1).astype(bf16)),
    }
    cvec = (W2.astype(np.float64) @ ln_b.astype(np.float64)
            + b2.astype(np.float64)).astype(f32)
    return common, kv.T.astype(f32), b1c, cvec


def kernel(x, W_down, b_down, W1, b1, ln_g, ln_b, W2, b2):
    x = np.asarray(x)
    common, kTfull, b1cfull, cvec = host_prep(
        x, np.asarray(W_down), np.asarray(b_down), np.asarray(W1),
        np.asarray(b1), np.asarray(ln_g), np.asarray(ln_b), np.asarray(W2),
        np.asarray(b2))

    nc = bacc.Bacc("TRN2")
    _build(nc)
    nc.finalize()

    in_maps = []
    for core in range(NCORES):
        m = dict(common)
        i0 = core * ROWS
        m["kTin"] = np.ascontiguousarray(kTfull[:, i0:i0 + ROWS])
        m["b1cTin"] = np.ascontiguousarray(
            b1cfull[:, i0:i0 + ROWS].T.reshape(1, ROWS * P).astype(
                ml_dtypes.bfloat16))
        in_maps.append(m)

    trace = os.environ.get("KERNEL_TRACE", "0") == "1"
    res = run_bass_kernel_spmd(nc, in_maps, core_ids=list(range(NCORES)),
                               trace=trace)
    if trace and res.exec_time_ns is not None:
        print(f"HW exec time: {res.exec_time_ns} ns")

    # host tail: LN scale + bias, transpose to [i, j, nb]
    outs = []
    for c in range(NCORES):
        dA = res.results[c]["dev_out"].astype(np.float32)  # [66, ROWS, L]
        po = dA[0:64]                                      # [64, ROWS, L]
        mu = dA[64] * np.float32(1.0 / 128.0)              # [ROWS, L]
        m2 = dA[65] * np.float32(1.0 / 128.0)
        var = m2 - mu * mu
        r = 1.0 / np.sqrt(var + np.float32(EPS))           # [ROWS, L]
        out = po.transpose(1, 2, 0) * r[:, :, None] + cvec[None, None, :]
        outs.append(out.astype(np.float32))
    full = np.concatenate(outs, axis=0)  # [768, 768, 64]
    return full[None].astype(np.float32)


# revision 36
# speedup vs baseline: 1.3238x; 1.3238x over previous
"""Trainium2 Bass kernel for nn_PairwisePredictionHead.

Math (reference):
  xd = x @ W_down.T + b_down             # [L, 128]
  q, k = xd[:, :64], xd[:, 64:]
  h[i,j,:] = W1p @ (q_j*k_i) + W1d @ (q_j - k_i) + b1    # [L, L, 128]
  g = gelu_exact(h)
  out = W2 @ LN(g) + b2                   # [L, L, 64]

Sharding: row-shard i across 8 cores (96 rows each). Each core gets the full
q-side (all 768 j) plus its own 96 k-rows; cores are independent (no
collectives), outputs concatenated on host.

Host precomputes the (tiny) downprojection q/k and per-row gelu bias
b1c = b1 - W1d@k_i; the device runs only the pairwise part.

Device (per core, per i; all matmuls bf16, 512/256-col splits for PSUM banks;
1-deep software pipeline so mm1(i+1) fills the PE while ACT/DVE finish i):
  - lhsT_i = [[W1pT * k_i[:,None]] ; W1dT]  (top half rebuilt per i on DVE)
  - p1[h, j]   = lhsT_i.T @ [q.T; q.T]            (PE, N=768)
  - g  = Gelu(p1 + b1c_i)                         (ACT, bf16 out)
  - g2 = g*g                                      (DVE, bf16)
  - pA[0:66, j] = [W2z.T*ln_g | ones | 0].T @ g   (PE; row 64 = sum_h g)
  - pA[64:66,j] += [0 | ones].T @ g2              (PE; row 65 = sum_h g^2)
  - copy pA[0:66] -> o_sb bf16 (DVE cols 0:CSPL, ACT the rest), batched DMA

Host tail (vectorized numpy): mu = Sg/128, var = Sg2/128 - mu^2,
  r = rsqrt(var+eps), out[i,j,:] = dev_out[:,i,j]*r + (W2@ln_b + b2).
W2z rows are zero-meaned so the matmul absorbs LN's mean subtraction
(w.(g-mu) == (w-mean(w)).g).
"""

import os
from contextlib import ExitStack

import numpy as np
import ml_dtypes

import concourse.bass as bass
import concourse.mybir as mybir
import concourse.tile as tile
from concourse import bacc
from concourse.bass_utils import run_bass_kernel_spmd

F32 = mybir.dt.float32
BF16 = mybir.dt.bfloat16
ALU = mybir.AluOpType
AF = mybir.ActivationFunctionType

B, L, D = 1, 768, 1024
DP, H, NB = 128, 128, 64
NCORES = 8
ROWS = L // NCORES  # 96 pair-grid rows per core
P = 128
EPS = 1e-5


def _build(nc):
    qqin = nc.dram_tensor("qqin", [P, L], BF16, kind="ExternalInput")
    kTin = nc.dram_tensor("kTin", [64, ROWS], F32, kind="ExternalInput")
    b1cin = nc.dram_tensor("b1cin", [P, ROWS], F32, kind="ExternalInput")
    W1pT = nc.dram_tensor("W1pT", [64, P], BF16, kind="ExternalInput")
    W1dT = nc.dram_tensor("W1dT", [64, P], BF16, kind="ExternalInput")
    W2A = nc.dram_tensor("W2A", [P, 66], BF16, kind="ExternalInput")
    onesc = nc.dram_tensor("onesc", [P, 2], BF16, kind="ExternalInput")
    # partition-major: rows 0:64 = W2z@g, row 64 = sum(g), row 65 = sum(g^2)
    dev_out = nc.dram_tensor("dev_out", [66, ROWS, L], BF16,
                             kind="ExternalOutput")

    with tile.TileContext(nc) as tc, ExitStack() as ctx:
        const = ctx.enter_context(tc.tile_pool(name="const", bufs=1))
        work = ctx.enter_context(tc.tile_pool(name="work", bufs=6))
        pp1 = ctx.enter_context(tc.tile_pool(name="pp1", bufs=2, space="PSUM"))
        ppA = ctx.enter_context(tc.tile_pool(name="ppA", bufs=2, space="PSUM"))

        # ---- constants into SBUF ----
        qq = const.tile([P, L], BF16)
        nc.sync.dma_start(out=qq, in_=qqin[:])
        kT_sb = const.tile([64, ROWS], F32)
        nc.sync.dma_start(out=kT_sb, in_=kTin[:])
        b1c = const.tile([P, ROWS], F32)
        nc.sync.dma_start(out=b1c, in_=b1cin[:])
        W1pT_sb = const.tile([64, P], BF16)
        nc.sync.dma_start(out=W1pT_sb, in_=W1pT[:])
        W2A_sb = const.tile([P, 66], BF16)
        nc.sync.dma_start(out=W2A_sb, in_=W2A[:])
        ones_sb = const.tile([P, 2], BF16)
        nc.sync.dma_start(out=ones_sb, in_=onesc[:])

        # persistent W1 stationary pair-tiles: each holds lhsT for two
        # consecutive i's side by side (bottom halves static = W1d.T)
        lhsT_t = [const.tile([P, 2, P], BF16, tag=f"lhsT{t}", name=f"lhsT{t}")
                  for t in range(2)]
        for t in range(2):
            for sl in range(2):
                nc.sync.dma_start(out=lhsT_t[t][64:128, sl, :], in_=W1dT[:])

        # ---- main loop (1-deep software pipeline) ----
        DB = 8      # i's per output-DMA batch
        CSPL = 464  # psum->SBUF copy column split (DVE | ACT)
        o_sb = None

        def build_lt(i):
            # build lhsT tops for the pair (i, i+1) in one DVE op:
            # [64, 2, 128] = W1pT[64, 1, 128] * k[:, i:i+2, None]
            pt = lhsT_t[(i // 2) % 2]
            n = min(2, ROWS - i)
            nc.vector.tensor_tensor(
                pt[0:64, 0:n, :],
                W1pT_sb[:, None, :].broadcast_to([64, n, P]),
                kT_sb[:, i:i + n, None].broadcast_to([64, n, P]),
                ALU.mult)
            return pt

        def mm1(i, lt):
            p1 = pp1.tile([P, L], F32, tag="p1", name="p1")
            nc.tensor.matmul(p1[:, 0:512], lt, qq[:, 0:512],
                             start=True, stop=True)
            nc.tensor.matmul(p1[:, 512:768], lt, qq[:, 512:768],
                             start=True, stop=True)
            return p1

        def copy_out(pc, ci):
            # lagged psum->SBUF copy of iteration ci (runs during ci+1 so
            # ACT/DVE never stall waiting on that iteration's mm2)
            nonlocal o_sb
            bi = ci % DB
            if bi == 0:
                o_sb = work.tile([66, DB, L], BF16, tag="osb", name="osb")
            nc.vector.tensor_copy(o_sb[:, bi, 0:CSPL], pc[0:66, 0:CSPL])
            nc.scalar.copy(o_sb[:, bi, CSPL:L], pc[0:66, CSPL:L])
            if bi == DB - 1:
                i0 = ci - (DB - 1)
                nc.sync.dma_start(out=dev_out[:, i0:i0 + DB, :], in_=o_sb)

        ltp = build_lt(0)
        p1 = mm1(0, ltp[:, 0, :])
        prev = None
        for ii in range(ROWS):
            g = work.tile([P, L], BF16, tag="g", name="g")
            nc.scalar.activation(g, p1, AF.Gelu, bias=b1c[:, ii:ii + 1])

            if ii + 1 < ROWS:
                if (ii + 1) % 2 == 0:
                    ltp = build_lt(ii + 1)
                p1 = mm1(ii + 1, ltp[:, (ii + 1) % 2, :])

            g2 = work.tile([P, L], BF16, tag="g2", name="g2")
            nc.vector.tensor_mul(g2, g, g)

            if prev is not None:
                copy_out(*prev)

            pA = ppA.tile([P, L], F32, tag="pA", name="pA")
            # MM-A first (needs only g): W2A = [W2z*ln_g | ones | zeros]
            # fills rows 0:66, with sum(g) on row 64 and row 65 zeroed.
            # MM-B then accumulates sum(g^2) into row 65 (start=False onto
            # MM-A's closed accumulation group; row 64 += 0).
            nc.tensor.matmul(pA[0:66, 0:512], W2A_sb, g[:, 0:512],
                             start=True, stop=True)
            nc.tensor.matmul(pA[0:66, 512:768], W2A_sb, g[:, 512:768],
                             start=True, stop=True)
            nc.tensor.matmul(pA[64:66, 0:512], ones_sb, g2[:, 0:512],
                             start=False, stop=True, skip_group_check=True)
            nc.tensor.matmul(pA[64:66, 512:768], ones_sb, g2[:, 512:768],
                             start=False, stop=True, skip_group_check=True)

            prev = (pA, ii)
        copy_out(*prev)


def host_prep(x, W_down, b_down, W1, b1, ln_g, ln_b, W2, b2):
    f32 = np.float32
    bf16 = ml_dtypes.bfloat16
    # downprojection on host (tiny): xd = x @ W_down.T + b_down
    xd = x[0].astype(f32) @ W_down.astype(f32).T + b_down.astype(f32)
    qv = xd[:, :64]        # [L, 64]
    kv = xd[:, 64:]        # [L, 64]
    qqf = np.concatenate([qv.T, qv.T], axis=0)  # [128, L]
    W1d = W1[:, 64:].astype(f32)
    # b1c[:, i] = b1 - W1d @ k_i
    b1c = b1.astype(f32)[:, None] - W1d @ kv.T.astype(f32)  # [128, L]

    W2g = W2.astype(np.float64) * ln_g.astype(np.float64)[None, :]
    W2z = W2g - W2g.mean(axis=1, keepdims=True)  # zero-mean rows absorb LN mu
    W2A = np.concatenate([W2z.T, np.ones((P, 1)), np.zeros((P, 1))],
                         axis=1)  # [128, 66]
    common = {
        "qqin": np.ascontiguousarray(qqf.astype(bf16)),
        "W1pT": np.ascontiguousarray(W1[:, :64].T.astype(bf16)),
        "W1dT": np.ascontiguousarray(W1[:, 64:].T.astype(bf16)),
        "W2A": np.ascontiguousarray(W2A.astype(bf16)),
        "onesc": np.ascontiguousarray(
            np.concatenate([np.zeros((P, 1)), np.ones((P, 1))],
                           axis=1).astype(bf16)),
    }
    cvec = (W2.astype(np.float64) @ ln_b.astype(np.float64)
            + b2.astype(np.float64)).astype(f32)
    return common, kv.T.astype(f32), b1c, cvec


def kernel(x, W_down, b_down, W1, b1, ln_g, ln_b, W2, b2):
    x = np.asarray(x)
    common, kTfull, b1cfull, cvec = host_prep(
        x, np.asarray(W_down), np.asarray(b_down), np.asarray(W1),
        np.asarray(b1), np.asarray(ln_g), np.asarray(ln_b), np.asarray(W2),
        np.asarray(b2))

    nc = bacc.Bacc("TRN2")
    _build(nc)
    nc.finalize()

    in_maps = []
    for core in range(NCORES):
        m = dict(common)
        i0 = core * ROWS
        m["kTin"] = np.ascontiguousarray(kTfull[:, i0:i0 + ROWS])
        m["b1cin"] = np.ascontiguousarray(b1cfull[:, i0:i0 + ROWS])
        in_maps.append(m)

    trace = os.environ.get("KERNEL_TRACE", "0") == "1"
    res = run_bass_kernel_spmd(nc, in_maps, core_ids=list(range(NCORES)),
                               trace=trace)
    if trace and res.exec_time_ns is not None:
        print(f"HW exec time: {res.exec_time_ns} ns")

    # host tail: LN scale + bias, transpose to [i, j, nb]
    outs = []
    for c in range(NCORES):
        dA = res.results[c]["dev_out"].astype(np.float32)  # [66, ROWS, L]
        po = dA[0:64]                                      # [64, ROWS, L]
        mu = dA[64] * np.float32(1.0 / 128.0)              # [ROWS, L]
        m2 = dA[65] * np.float32(1.0 / 128.0)
        var = m2 - mu * mu
        r = 1.0 / np.sqrt(var + np.float32(EPS))           # [ROWS, L]
        out = po.transpose(1, 2, 0) * r[:, :, None] + cvec[None, None, :]
        outs.append(out.astype(np.float32))
    full = np.concatenate(outs, axis=0)  # [768, 768, 64]
    return full[None].astype(np.float32)


# revision 37
# speedup vs baseline: 1.3364x; 1.0095x over previous
"""Trainium2 Bass kernel for nn_PairwisePredictionHead.

Math (reference):
  xd = x @ W_down.T + b_down             # [L, 128]
  q, k = xd[:, :64], xd[:, 64:]
  h[i,j,:] = W1p @ (q_j*k_i) + W1d @ (q_j - k_i) + b1    # [L, L, 128]
  g = gelu_exact(h)
  out = W2 @ LN(g) + b2                   # [L, L, 64]

Sharding: row-shard i across 8 cores (96 rows each). Each core gets the full
q-side (all 768 j) plus its own 96 k-rows; cores are independent (no
collectives), outputs concatenated on host.

Host precomputes the (tiny) downprojection q/k and per-row gelu bias
b1c = b1 - W1d@k_i; the device runs only the pairwise part.

Device (per core, per i; all matmuls bf16, 512/256-col splits for PSUM banks;
1-deep software pipeline so mm1(i+1) fills the PE while ACT/DVE finish i):
  - lhsT_i = [[W1pT * k_i[:,None]] ; W1dT]  (top half rebuilt per i on DVE)
  - p1[h, j]   = lhsT_i.T @ [q.T; q.T]            (PE, N=768)
  - g  = Gelu(p1 + b1c_i)                         (ACT, bf16 out)
  - g2 = g*g                                      (DVE, bf16)
  - pA[0:66, j] = [W2z.T*ln_g | ones | 0].T @ g   (PE; row 64 = sum_h g)
  - pA[64:66,j] += [0 | ones].T @ g2              (PE; row 65 = sum_h g^2)
  - copy pA[0:66] -> o_sb bf16 (DVE cols 0:CSPL, ACT the rest), batched DMA

Host tail (vectorized numpy): mu = Sg/128, var = Sg2/128 - mu^2,
  r = rsqrt(var+eps), out[i,j,:] = dev_out[:,i,j]*r + (W2@ln_b + b2).
W2z rows are zero-meaned so the matmul absorbs LN's mean subtraction
(w.(g-mu) == (w-mean(w)).g).
"""

import os
from contextlib import ExitStack

import numpy as np
import ml_dtypes

import concourse.bass as bass
import concourse.mybir as mybir
import concourse.tile as tile
from concourse import bacc
from concourse.bass_utils import run_bass_kernel_spmd

F32 = mybir.dt.float32
BF16 = mybir.dt.bfloat16
ALU = mybir.AluOpType
AF = mybir.ActivationFunctionType

B, L, D = 1, 768, 1024
DP, H, NB = 128, 128, 64
NCORES = 8
ROWS = L // NCORES  # 96 pair-grid rows per core
P = 128
EPS = 1e-5


def _build(nc):
    qqin = nc.dram_tensor("qqin", [P, L], BF16, kind="ExternalInput")
    kTin = nc.dram_tensor("kTin", [64, ROWS], F32, kind="ExternalInput")
    b1cin = nc.dram_tensor("b1cin", [P, ROWS], F32, kind="ExternalInput")
    W1pT = nc.dram_tensor("W1pT", [64, P], BF16, kind="ExternalInput")
    W1dT = nc.dram_tensor("W1dT", [64, P], BF16, kind="ExternalInput")
    W2A = nc.dram_tensor("W2A", [P, 66], BF16, kind="ExternalInput")
    onesc = nc.dram_tensor("onesc", [P, 2], BF16, kind="ExternalInput")
    # partition-major: rows 0:64 = W2z@g, row 64 = sum(g), row 65 = sum(g^2)
    dev_out = nc.dram_tensor("dev_out", [66, ROWS, L], BF16,
                             kind="ExternalOutput")

    with tile.TileContext(nc) as tc, ExitStack() as ctx:
        const = ctx.enter_context(tc.tile_pool(name="const", bufs=1))
        work = ctx.enter_context(tc.tile_pool(name="work", bufs=6))
        pp1 = ctx.enter_context(tc.tile_pool(name="pp1", bufs=2, space="PSUM"))
        ppA = ctx.enter_context(tc.tile_pool(name="ppA", bufs=2, space="PSUM"))

        # ---- constants into SBUF ----
        qq = const.tile([P, L], BF16)
        nc.sync.dma_start(out=qq, in_=qqin[:])
        kT_sb = const.tile([64, ROWS], F32)
        nc.sync.dma_start(out=kT_sb, in_=kTin[:])
        b1c = const.tile([P, ROWS], F32)
        nc.sync.dma_start(out=b1c, in_=b1cin[:])
        W1pT_sb = const.tile([64, P], BF16)
        nc.sync.dma_start(out=W1pT_sb, in_=W1pT[:])
        W2A_sb = const.tile([P, 66], BF16)
        nc.sync.dma_start(out=W2A_sb, in_=W2A[:])
        ones_sb = const.tile([P, 2], BF16)
        nc.sync.dma_start(out=ones_sb, in_=onesc[:])

        # persistent W1 stationary pair-tiles: each holds lhsT for two
        # consecutive i's side by side (bottom halves static = W1d.T)
        lhsT_t = [const.tile([P, 2, P], BF16, tag=f"lhsT{t}", name=f"lhsT{t}")
                  for t in range(2)]
        for t in range(2):
            for sl in range(2):
                nc.sync.dma_start(out=lhsT_t[t][64:128, sl, :], in_=W1dT[:])

        # ---- main loop (1-deep software pipeline) ----
        DB = 8      # i's per output-DMA batch
        CSPL = 464  # psum->SBUF copy column split (DVE | ACT)
        o_sb = None

        def build_lt(i):
            # build lhsT tops for the pair (i, i+1) in one DVE op:
            # [64, 2, 128] = W1pT[64, 1, 128] * k[:, i:i+2, None]
            pt = lhsT_t[(i // 2) % 2]
            n = min(2, ROWS - i)
            nc.vector.tensor_tensor(
                pt[0:64, 0:n, :],
                W1pT_sb[:, None, :].broadcast_to([64, n, P]),
                kT_sb[:, i:i + n, None].broadcast_to([64, n, P]),
                ALU.mult)
            return pt

        def mm1(i, lt):
            p1 = pp1.tile([P, L], F32, tag="p1", name="p1")
            nc.tensor.matmul(p1[:, 0:512], lt, qq[:, 0:512],
                             start=True, stop=True)
            nc.tensor.matmul(p1[:, 512:768], lt, qq[:, 512:768],
                             start=True, stop=True)
            return p1

        def copy_out(pc, ci):
            # lagged psum->SBUF copy of iteration ci (runs during ci+1 so
            # ACT/DVE never stall waiting on that iteration's mm2)
            nonlocal o_sb
            bi = ci % DB
            if bi == 0:
                o_sb = work.tile([66, DB, L], BF16, tag="osb", name="osb")
            nc.vector.tensor_copy(o_sb[:, bi, 0:CSPL], pc[0:66, 0:CSPL])
            nc.scalar.copy(o_sb[:, bi, CSPL:L], pc[0:66, CSPL:L])
            if bi == DB - 1:
                i0 = ci - (DB - 1)
                nc.sync.dma_start(out=dev_out[:, i0:i0 + DB, :], in_=o_sb)

        ltp = build_lt(0)
        p1 = mm1(0, ltp[:, 0, :])
        prev = None
        for ii in range(ROWS):
            g = work.tile([P, L], BF16, tag="g", name="g")
            nc.scalar.activation(g, p1, AF.Gelu, bias=b1c[:, ii:ii + 1])

            if ii + 1 < ROWS:
                if (ii + 1) % 2 == 0:
                    ltp = build_lt(ii + 1)
                p1 = mm1(ii + 1, ltp[:, (ii + 1) % 2, :])

            g2 = work.tile([P, L], BF16, tag="g2", name="g2")
            nc.vector.tensor_mul(g2, g, g)

            if prev is not None:
                copy_out(*prev)

            pA = ppA.tile([P, L], F32, tag="pA", name="pA")
            # MM-A first (needs only g): W2A = [W2z*ln_g | ones | zeros]
            # fills rows 0:66, with sum(g) on row 64 and row 65 zeroed.
            # MM-B then accumulates sum(g^2) into row 65 (start=False onto
            # MM-A's closed accumulation group; row 64 += 0).
            nc.tensor.matmul(pA[0:66, 0:512], W2A_sb, g[:, 0:512],
                             start=True, stop=True)
            nc.tensor.matmul(pA[0:66, 512:768], W2A_sb, g[:, 512:768],
                             start=True, stop=True)
            nc.tensor.matmul(pA[64:66, 0:512], ones_sb, g2[:, 0:512],
                             start=False, stop=True, skip_group_check=True)
            nc.tensor.matmul(pA[64:66, 512:768], ones_sb, g2[:, 512:768],
                             start=False, stop=True, skip_group_check=True)

            prev = (pA, ii)
        copy_out(*prev)


def host_prep(x, W_down, b_down, W1, b1, ln_g, ln_b, W2, b2):
    f32 = np.float32
    bf16 = ml_dtypes.bfloat16
    # downprojection on host (tiny): xd = x @ W_down.T + b_down
    xd = x[0].astype(f32) @ W_down.astype(f32).T + b_down.astype(f32)
    qv = xd[:, :64]        # [L, 64]
    kv = xd[:, 64:]        # [L, 64]
    qqf = np.concatenate([qv.T, qv.T], axis=0)  # [128, L]
    W1d = W1[:, 64:].astype(f32)
    # b1c[:, i] = b1 - W1d @ k_i
    b1c = b1.astype(f32)[:, None] - W1d @ kv.T.astype(f32)  # [128, L]

    W2g = W2.astype(np.float64) * ln_g.astype(np.float64)[None, :]
    W2z = W2g - W2g.mean(axis=1, keepdims=True)  # zero-mean rows absorb LN mu
    W2A = np.concatenate([W2z.T, np.ones((P, 1)), np.zeros((P, 1))],
                         axis=1)  # [128, 66]
    common = {
        "qqin": np.ascontiguousarray(qqf.astype(bf16)),
        "W1pT": np.ascontiguousarray(W1[:, :64].T.astype(bf16)),
        "W1dT": np.ascontiguousarray(W1[:, 64:].T.astype(bf16)),
        "W2A": np.ascontiguousarray(W2A.astype(bf16)),
        "onesc": np.ascontiguousarray(
            np.concatenate([np.zeros((P, 1)), np.ones((P, 1))],
                           axis=1).astype(bf16)),
    }
    cvec = (W2.astype(np.float64) @ ln_b.astype(np.float64)
            + b2.astype(np.float64)).astype(f32)
    return common, kv.T.astype(f32), b1c, cvec


def _install_trace_shims():
    """Enable NTFF profiling under axon on images whose antenv lacks
    axon_hooks. Returns True if tracing is usable."""
    try:
        import antenv.axon_hooks  # noqa: F401
        return True
    except ImportError:
        pass
    try:
        import sys
        import types

        import antenv
        from trn_agent_boot.trn_boot import _ntff_profile_via_ctypes

        mod = types.ModuleType("antenv.axon_hooks")
        mod._hook = _ntff_profile_via_ctypes("/opt/axon/libaxon_pjrt.so")
        mod.set_axon_ntff_profile_hook = lambda h: setattr(mod, "_hook", h)
        mod.get_axon_ntff_profile_hook = lambda: mod._hook
        sys.modules["antenv.axon_hooks"] = mod
        antenv.axon_hooks = mod

        import concourse.bass_utils as bu

        bu.upload_artifacts = lambda tmpdir: f"local://{tmpdir}"
        return True
    except Exception:
        return False


def kernel(x, W_down, b_down, W1, b1, ln_g, ln_b, W2, b2):
    x = np.asarray(x)
    common, kTfull, b1cfull, cvec = host_prep(
        x, np.asarray(W_down), np.asarray(b_down), np.asarray(W1),
        np.asarray(b1), np.asarray(ln_g), np.asarray(ln_b), np.asarray(W2),
        np.asarray(b2))

    nc = bacc.Bacc("TRN2")
    _build(nc)
    nc.finalize()

    in_maps = []
    for core in range(NCORES):
        m = dict(common)
        i0 = core * ROWS
        m["kTin"] = np.ascontiguousarray(kTfull[:, i0:i0 + ROWS])
        m["b1cin"] = np.ascontiguousarray(b1cfull[:, i0:i0 + ROWS])
        in_maps.append(m)

    trace = os.environ.get("KERNEL_TRACE", "0") == "1"
    if trace:
        trace = _install_trace_shims()
    res = run_bass_kernel_spmd(nc, in_maps, core_ids=list(range(NCORES)),
                               trace=trace)
    if trace and res.exec_time_ns is not None:
        print(f"HW exec time: {res.exec_time_ns} ns")

    # host tail: LN scale + bias, transpose to [i, j, nb]
    outs = []
    for c in range(NCORES):
        dA = res.results[c]["dev_out"].astype(np.float32)  # [66, ROWS, L]
        po = dA[0:64]                                      # [64, ROWS, L]
        mu = dA[64] * np.float32(1.0 / 128.0)              # [ROWS, L]
        m2 = dA[65] * np.float32(1.0 / 128.0)
        var = m2 - mu * mu
        r = 1.0 / np.sqrt(var + np.float32(EPS))           # [ROWS, L]
        out = po.transpose(1, 2, 0) * r[:, :, None] + cvec[None, None, :]
        outs.append(out.astype(np.float32))
    full = np.concatenate(outs, axis=0)  # [768, 768, 64]
    return full[None].astype(np.float32)


# revision 38
# speedup vs baseline: 1.3657x; 1.0219x over previous
"""Trainium2 Bass kernel for nn_PairwisePredictionHead.

Math (reference):
  xd = x @ W_down.T + b_down             # [L, 128]
  q, k = xd[:, :64], xd[:, 64:]
  h[i,j,:] = W1p @ (q_j*k_i) + W1d @ (q_j - k_i) + b1    # [L, L, 128]
  g = gelu_exact(h)
  out = W2 @ LN(g) + b2                   # [L, L, 64]

Sharding: row-shard i across 8 cores (96 rows each). Each core gets the full
q-side (all 768 j) plus its own 96 k-rows; cores are independent (no
collectives), outputs concatenated on host.

Host precomputes the (tiny) downprojection q/k and per-row gelu bias
b1c = b1 - W1d@k_i; the device runs only the pairwise part.

Device (per core, per i; all matmuls bf16, 512/256-col splits for PSUM banks;
1-deep software pipeline so mm1(i+1) fills the PE while ACT/DVE finish i):
  - lhsT_i = [[W1pT * k_i[:,None]] ; W1dT]  (top half rebuilt per i on DVE)
  - p1[h, j]   = lhsT_i.T @ [q.T; q.T]            (PE, N=768)
  - g  = Gelu(p1 + b1c_i)                         (ACT, bf16 out)
  - g2 = g*g                                      (DVE, bf16)
  - pA[0:66, j] = [W2z.T*ln_g | ones | 0].T @ g   (PE; row 64 = sum_h g)
  - pA[64:66,j] += [0 | ones].T @ g2              (PE; row 65 = sum_h g^2)
  - copy pA[0:66] -> o_sb bf16 (DVE cols 0:CSPL, ACT the rest), batched DMA

Host tail (vectorized numpy): mu = Sg/128, var = Sg2/128 - mu^2,
  r = rsqrt(var+eps), out[i,j,:] = dev_out[:,i,j]*r + (W2@ln_b + b2).
W2z rows are zero-meaned so the matmul absorbs LN's mean subtraction
(w.(g-mu) == (w-mean(w)).g).
"""

import os
from contextlib import ExitStack

import numpy as np
import ml_dtypes

import concourse.bass as bass
import concourse.mybir as mybir
import concourse.tile as tile
from concourse import bacc
from concourse.bass_utils import run_bass_kernel_spmd

F32 = mybir.dt.float32
BF16 = mybir.dt.bfloat16
ALU = mybir.AluOpType
AF = mybir.ActivationFunctionType

B, L, D = 1, 768, 1024
DP, H, NB = 128, 128, 64
NCORES = 8
ROWS = L // NCORES  # 96 pair-grid rows per core
P = 128
EPS = 1e-5


def _build(nc):
    qqin = nc.dram_tensor("qqin", [P, L], BF16, kind="ExternalInput")
    kTin = nc.dram_tensor("kTin", [64, ROWS], F32, kind="ExternalInput")
    b1cin = nc.dram_tensor("b1cin", [P, ROWS], F32, kind="ExternalInput")
    W1pT = nc.dram_tensor("W1pT", [64, P], BF16, kind="ExternalInput")
    W1dT2 = nc.dram_tensor("W1dT2", [64, 2, P], BF16, kind="ExternalInput")
    W2A = nc.dram_tensor("W2A", [P, 66], BF16, kind="ExternalInput")
    onesc = nc.dram_tensor("onesc", [P, 2], BF16, kind="ExternalInput")
    # partition-major: rows 0:64 = W2z@g, row 64 = sum(g), row 65 = sum(g^2)
    dev_out = nc.dram_tensor("dev_out", [66, ROWS, L], BF16,
                             kind="ExternalOutput")

    with tile.TileContext(nc) as tc, ExitStack() as ctx:
        const = ctx.enter_context(tc.tile_pool(name="const", bufs=1))
        work = ctx.enter_context(tc.tile_pool(name="work", bufs=6))
        pp1 = ctx.enter_context(tc.tile_pool(name="pp1", bufs=2, space="PSUM"))
        ppA = ctx.enter_context(tc.tile_pool(name="ppA", bufs=2, space="PSUM"))

        # ---- constants into SBUF (mm1(0)-critical loads first, issued on
        # two queues so the ~600ns doorbells overlap) ----
        kT_sb = const.tile([64, ROWS], F32)
        W1pT_sb = const.tile([64, P], BF16)
        qq = const.tile([P, L], BF16)
        b1c = const.tile([P, ROWS], F32)
        W2A_sb = const.tile([P, 66], BF16)
        ones_sb = const.tile([P, 2], BF16)
        lhsT_t = [const.tile([P, 2, P], BF16, tag=f"lhsT{t}", name=f"lhsT{t}")
                  for t in range(2)]
        nc.sync.dma_start(out=kT_sb, in_=kTin[:])
        nc.gpsimd.dma_start(out=W1pT_sb, in_=W1pT[:])
        nc.sync.dma_start(out=qq, in_=qqin[:])
        nc.gpsimd.dma_start(out=lhsT_t[0][64:128, :, :], in_=W1dT2[:])
        nc.sync.dma_start(out=lhsT_t[1][64:128, :, :], in_=W1dT2[:])
        nc.gpsimd.dma_start(out=b1c, in_=b1cin[:])
        nc.sync.dma_start(out=W2A_sb, in_=W2A[:])
        nc.gpsimd.dma_start(out=ones_sb, in_=onesc[:])

        # ---- main loop (1-deep software pipeline) ----
        CSPL = 464  # psum->SBUF copy column split (DVE | ACT)
        # output-DMA batches: big in steady state, small at the end so the
        # final DMA tail is short; each DMA split into two partition halves
        # to run on two queues
        BATCHES = [8] * 11 + [4, 2, 1, 1]
        batch_of = {}
        b0 = 0
        for blen in BATCHES:
            for off in range(blen):
                batch_of[b0 + off] = (b0, blen, off)
            b0 += blen
        o_sb = None

        def build_lt(i):
            # build lhsT tops for the pair (i, i+1) in one DVE op:
            # [64, 2, 128] = W1pT[64, 1, 128] * k[:, i:i+2, None]
            pt = lhsT_t[(i // 2) % 2]
            n = min(2, ROWS - i)
            nc.vector.tensor_tensor(
                pt[0:64, 0:n, :],
                W1pT_sb[:, None, :].broadcast_to([64, n, P]),
                kT_sb[:, i:i + n, None].broadcast_to([64, n, P]),
                ALU.mult)
            return pt

        def mm1(i, lt):
            p1 = pp1.tile([P, L], F32, tag="p1", name="p1")
            nc.tensor.matmul(p1[:, 0:512], lt, qq[:, 0:512],
                             start=True, stop=True)
            nc.tensor.matmul(p1[:, 512:768], lt, qq[:, 512:768],
                             start=True, stop=True)
            return p1

        def copy_out(pc, ci):
            # lagged psum->SBUF copy of iteration ci (runs during ci+1 so
            # ACT/DVE never stall waiting on that iteration's mm2)
            nonlocal o_sb
            i0, blen, bi = batch_of[ci]
            if bi == 0:
                o_sb = work.tile([66, blen, L], BF16, tag="osb", name="osb")
            nc.vector.tensor_copy(o_sb[:, bi, 0:CSPL], pc[0:66, 0:CSPL])
            nc.scalar.copy(o_sb[:, bi, CSPL:L], pc[0:66, CSPL:L])
            if bi == blen - 1:
                nc.sync.dma_start(out=dev_out[0:33, i0:i0 + blen, :],
                                  in_=o_sb[0:33])
                nc.gpsimd.dma_start(out=dev_out[33:66, i0:i0 + blen, :],
                                    in_=o_sb[33:66])

        ltp = build_lt(0)
        p1 = mm1(0, ltp[:, 0, :])
        prev = None
        for ii in range(ROWS):
            g = work.tile([P, L], BF16, tag="g", name="g")
            nc.scalar.activation(g, p1, AF.Gelu, bias=b1c[:, ii:ii + 1])

            if ii + 1 < ROWS:
                if (ii + 1) % 2 == 0:
                    ltp = build_lt(ii + 1)
                p1 = mm1(ii + 1, ltp[:, (ii + 1) % 2, :])

            g2 = work.tile([P, L], BF16, tag="g2", name="g2")
            nc.vector.tensor_mul(g2, g, g)

            if prev is not None:
                copy_out(*prev)

            pA = ppA.tile([P, L], F32, tag="pA", name="pA")
            # MM-A first (needs only g): W2A = [W2z*ln_g | ones | zeros]
            # fills rows 0:66, with sum(g) on row 64 and row 65 zeroed.
            # MM-B then accumulates sum(g^2) into row 65 (start=False onto
            # MM-A's closed accumulation group; row 64 += 0).
            nc.tensor.matmul(pA[0:66, 0:512], W2A_sb, g[:, 0:512],
                             start=True, stop=True)
            nc.tensor.matmul(pA[0:66, 512:768], W2A_sb, g[:, 512:768],
                             start=True, stop=True)
            nc.tensor.matmul(pA[64:66, 0:512], ones_sb, g2[:, 0:512],
                             start=False, stop=True, skip_group_check=True)
            nc.tensor.matmul(pA[64:66, 512:768], ones_sb, g2[:, 512:768],
                             start=False, stop=True, skip_group_check=True)

            prev = (pA, ii)
        copy_out(*prev)


def host_prep(x, W_down, b_down, W1, b1, ln_g, ln_b, W2, b2):
    f32 = np.float32
    bf16 = ml_dtypes.bfloat16
    # downprojection on host (tiny): xd = x @ W_down.T + b_down
    xd = x[0].astype(f32) @ W_down.astype(f32).T + b_down.astype(f32)
    qv = xd[:, :64]        # [L, 64]
    kv = xd[:, 64:]        # [L, 64]
    qqf = np.concatenate([qv.T, qv.T], axis=0)  # [128, L]
    W1d = W1[:, 64:].astype(f32)
    # b1c[:, i] = b1 - W1d @ k_i
    b1c = b1.astype(f32)[:, None] - W1d @ kv.T.astype(f32)  # [128, L]

    W2g = W2.astype(np.float64) * ln_g.astype(np.float64)[None, :]
    W2z = W2g - W2g.mean(axis=1, keepdims=True)  # zero-mean rows absorb LN mu
    W2A = np.concatenate([W2z.T, np.ones((P, 1)), np.zeros((P, 1))],
                         axis=1)  # [128, 66]
    common = {
        "qqin": np.ascontiguousarray(qqf.astype(bf16)),
        "W1pT": np.ascontiguousarray(W1[:, :64].T.astype(bf16)),
        "W1dT2": np.ascontiguousarray(
            np.repeat(W1[:, 64:].T.astype(bf16)[:, None, :], 2, axis=1)),
        "W2A": np.ascontiguousarray(W2A.astype(bf16)),
        "onesc": np.ascontiguousarray(
            np.concatenate([np.zeros((P, 1)), np.ones((P, 1))],
                           axis=1).astype(bf16)),
    }
    cvec = (W2.astype(np.float64) @ ln_b.astype(np.float64)
            + b2.astype(np.float64)).astype(f32)
    return common, kv.T.astype(f32), b1c, cvec


def _install_trace_shims():
    """Enable NTFF profiling under axon on images whose antenv lacks
    axon_hooks. Returns True if tracing is usable."""
    try:
        import antenv.axon_hooks  # noqa: F401
        return True
    except ImportError:
        pass
    try:
        import sys
        import types

        import antenv
        from trn_agent_boot.trn_boot import _ntff_profile_via_ctypes

        mod = types.ModuleType("antenv.axon_hooks")
        mod._hook = _ntff_profile_via_ctypes("/opt/axon/libaxon_pjrt.so")
        mod.set_axon_ntff_profile_hook = lambda h: setattr(mod, "_hook", h)
        mod.get_axon_ntff_profile_hook = lambda: mod._hook
        sys.modules["antenv.axon_hooks"] = mod
        antenv.axon_hooks = mod

        import concourse.bass_utils as bu

        bu.upload_artifacts = lambda tmpdir: f"local://{tmpdir}"
        return True
    except Exception:
        return False


def kernel(x, W_down, b_down, W1, b1, ln_g, ln_b, W2, b2):
    x = np.asarray(x)
    common, kTfull, b1cfull, cvec = host_prep(
        x, np.asarray(W_down), np.asarray(b_down), np.asarray(W1),
        np.asarray(b1), np.asarray(ln_g), np.asarray(ln_b), np.asarray(W2),
        np.asarray(b2))

    nc = bacc.Bacc("TRN2")
    _build(nc)
    nc.finalize()

    in_maps = []
    for core in range(NCORES):
        m = dict(common)
        i0 = core * ROWS
        m["kTin"] = np.ascontiguousarray(kTfull[:, i0:i0 + ROWS])
        m["b1cin"] = np.ascontiguousarray(b1cfull[:, i0:i0 + ROWS])
        in_maps.append(m)

    trace = os.environ.get("KERNEL_TRACE", "0") == "1"
    if trace:
        trace = _install_trace_shims()
    res = run_bass_kernel_spmd(nc, in_maps, core_ids=list(range(NCORES)),
                               trace=trace)
    if trace and res.exec_time_ns is not None:
        print(f"HW exec time: {res.exec_time_ns} ns")

    # host tail: LN scale + bias, transpose to [i, j, nb]
    outs = []
    for c in range(NCORES):
        dA = res.results[c]["dev_out"].astype(np.float32)  # [66, ROWS, L]
        po = dA[0:64]                                      # [64, ROWS, L]
        mu = dA[64] * np.float32(1.0 / 128.0)              # [ROWS, L]
        m2 = dA[65] * np.float32(1.0 / 128.0)
        var = m2 - mu * mu
        r = 1.0 / np.sqrt(var + np.float32(EPS))           # [ROWS, L]
        out = po.transpose(1, 2, 0) * r[:, :, None] + cvec[None, None, :]
        outs.append(out.astype(np.float32))
    full = np.concatenate(outs, axis=0)  # [768, 768, 64]
    return full[None].astype(np.float32)
